# revision 45
# baseline (speedup 1.0000x reference)
"""MultiHeadAttention (partial RoPE) Trainium2 Bass kernel — v3.

Sharding: 8 cores = 2 batches x 4 head-groups (4 heads each).
Each core computes a partial output (L, D) for its batch from its 4 heads;
the host sums the 4 partials per batch (the "all-reduce after o-proj").

Design (vs the naive decomposition; the PE cost is ~N output columns per
matmul regardless of contraction rows C or lhsT width M):
  - q/k stored as qcat/kcat [128 = 2 heads x (32 rot + 32 pass), pair, L]:
    QK^T per head is ONE C=64 matmul (rot+pass contracted together).
  - rotate_half = a PE permutation matmul (perm [128,128]) on the projected
    q/k instead of a second full D-contraction projection.
  - AV in natural orientation: out[lq-chunk 128, d] with lhsT = exp scores
    [lk, lq-chunk], rhs = v_h [lk, 64]: N=64 per matmul (vs N=512, M=65);
    softmax denominators via extra N=1 matmuls against a ones column.
  - denominators land as per-PARTITION scalars -> normalize is a cheap
    tensor_scalar/ACT-scale multiply (no DRAM round-trip / broadcast).
  - attn transposed to [hd, lq] via PE transpose (N=128) so the
    o-projection contracts C=128 (2 heads stacked).
  - softmax exp is the ACT-engine floor (~110us): ~43% of score tiles are
    computed instead on DVE/Pool with a refined Schraudolph fast-exp
    (int16(A*x+B) bitcast to bf16; product of a phase-shifted pair,
    ~+-1.9% rel err): y1 (PSUM read) on DVE, y2+mult on Pool/DVE.
    GPSIMD/Pool cannot access PSUM on hw - only DVE/ACT ops read PSUM.
  - per-head 1-bank PSUM score tiles (4 in flight) + AV matmuls lagged 20
    head-iterations behind QK/exp: hides exp latency; the kernel ends up
    PE-bound at ~84% occupancy.
  - phases: A = k/v projections; B = sweep (q projection for the next lq
    tile hidden inside); C = transpose + o-projection + bf16 output DMA.
"""

import sys

if "/opt/trn_rl_repo" not in sys.path:
    sys.path.insert(0, "/opt/trn_rl_repo")

from contextlib import ExitStack

import ml_dtypes
import numpy as np

import concourse.bass as bass
import concourse.mybir as mybir
from concourse import bacc
import concourse.tile as tile

B, L_FULL, D = 2, 2048, 1024
H, K = 16, 64
ROT = 32
HPC = 4  # heads per core
NP = 2  # head pairs per core
NCORES = 8

F32 = mybir.dt.float32
BF16 = mybir.dt.bfloat16
I16 = mybir.dt.int16
NPBF = ml_dtypes.bfloat16

LQ = 512  # lq tile
LK = 128  # lk tile (partition tile)
NC = D // 128  # contraction chunks for projections

# fast-exp (Schraudolph in bf16-bit domain, refined by a phase-shifted pair):
#   exp(x) ~= S((x+d)/2) * S((x-d)/2),  S(t) = bitcast_bf16(int16(A*t + B))
# ~+-1.9% relative error; used to offload part of the softmax exp from the
# ACT engine (the kernel's bottleneck) onto the otherwise-idle Pool/DVE.
_LOG2E = 1.4426950408889634
_FE_C = 0.054
_FE_D = 0.35
_FE_A = 2.0**7 * _LOG2E
_FE_B = 127.0 * 2**7 - _FE_C * 2**7
# exp engine shares per 128 iterations (ACT is ~2.5x faster per element
# than the 3-op fast-exp path on Pool/DVE); interleaved evenly.
import os

_EXP_SHARES = {
    "A": int(os.environ.get("EXP_A", "72")),
    "P": int(os.environ.get("EXP_P", "30")),
    "D": int(os.environ.get("EXP_D", "26")),
}
_HALF_EXP = os.environ.get("HALF_EXP", "0") == "1"
_PER_HEAD = os.environ.get("PER_HEAD", "1") == "1"
_PH_SHARES = {
    "A": int(os.environ.get("PH_A", "146")),
    "P": int(os.environ.get("PH_P", "74")),
    "D": int(os.environ.get("PH_D", "36")),
}


def _make_exp_sched(shares):
    tot = sum(shares.values())
    acc = {e: 0.0 for e in shares}
    sched = []
    for _ in range(tot):
        for e in acc:
            acc[e] += shares[e] / tot
        pick = max(acc, key=lambda e: acc[e])
        acc[pick] -= 1.0
        sched.append(pick)
    return sched


_EXP_SCHED = _make_exp_sched(_PH_SHARES if _PER_HEAD else _EXP_SHARES)


def _exp_engine(it):
    return _EXP_SCHED[it % len(_EXP_SCHED)]


def build_nc(L=L_FULL):
    """Build the single-core SPMD program. Returns nc."""
    nc = bacc.Bacc("TRN2", target_bir_lowering=False)

    NLQ = L // LQ  # lq tiles
    NLK = L // LK  # lk tiles
    NLQC = L // LK  # lq chunks (128)

    ACT_COPY = mybir.ActivationFunctionType.Copy
    ACT_EXP = mybir.ActivationFunctionType.Exp

    # ---- DRAM I/O (per-core shapes) ----
    xq = nc.dram_tensor("xq", [D, L], BF16, kind="ExternalInput")  # query[b].T
    xk = nc.dram_tensor("xk", [D, L], BF16, kind="ExternalInput")
    xv = nc.dram_tensor("xv", [D, L], BF16, kind="ExternalInput")
    wqm = nc.dram_tensor("wqm", [D, NP, 128], BF16, kind="ExternalInput")
    wkm = nc.dram_tensor("wkm", [D, NP, 128], BF16, kind="ExternalInput")
    wv = nc.dram_tensor("wv", [D, 256], BF16, kind="ExternalInput")
    wo = nc.dram_tensor("wo", [NP, 128, D], BF16, kind="ExternalInput")
    cost = nc.dram_tensor("cost", [128, L], BF16, kind="ExternalInput")
    sint = nc.dram_tensor("sint", [128, L], BF16, kind="ExternalInput")
    perm = nc.dram_tensor("perm", [128, 128], BF16, kind="ExternalInput")
    ident = nc.dram_tensor("ident", [128, 128], BF16, kind="ExternalInput")
    out = nc.dram_tensor("out", [L, D], BF16, kind="ExternalOutput")

    with tile.TileContext(nc) as tc, ExitStack() as ctx:
        consts = ctx.enter_context(tc.tile_pool(name="consts", bufs=1))
        persist = ctx.enter_context(tc.tile_pool(name="persist", bufs=1))

        # ---- load k-side consts + first x tiles up front, in parallel across
        # engines, so the PE can start ASAP. wqm/wo/ident are emitted at the
        # end of phase A so they don't block phase-A ACT evictions. ----
        wkm_s = consts.tile([128, NC, NP, 128], BF16, tag="wkm")
        nc.sync.dma_start(out=wkm_s[:], in_=wkm.rearrange("(c p) n m -> p c n m", p=128))
        perm_s = consts.tile([128, 128], BF16, tag="perm")
        nc.sync.dma_start(out=perm_s[:], in_=perm[:])
        wv_s = consts.tile([128, NC, 256], BF16, tag="wv")
        nc.scalar.dma_start(out=wv_s[:], in_=wv.rearrange("(c p) n -> p c n", p=128))
        cos_s = consts.tile([128, L], BF16, tag="cos")
        nc.scalar.dma_start(out=cos_s[:], in_=cost[:])
        sin_s = consts.tile([128, L], BF16, tag="sin")
        nc.scalar.dma_start(out=sin_s[:], in_=sint[:])
        wqm_s = consts.tile([128, NC, NP, 128], BF16, tag="wqm")
        wo_s = consts.tile([128, NP, D], BF16, tag="wo")
        ident_s = consts.tile([128, 128], BF16, tag="ident")

        # ---- persistent activations ----
        qcat = persist.tile([128, NP, L], BF16, tag="qcat")
        kcat = persist.tile([128, NP, L], BF16, tag="kcat")
        v_s = persist.tile([128, NLK, HPC, 65], BF16, tag="v")
        attnN = persist.tile([128, NLQC, HPC, K], BF16, tag="attnN")
        nc.vector.memset(v_s[:, :, :, 64:65], 1.0)

        # ================= phase A: k/v projections =================
        # (q projection runs inside the phase-B sweep, hidden under the
        # ACT-bound exp pipeline, using a single shared PSUM bank.)
        ev = ctx.enter_context(tc.tile_pool(name="ev", bufs=3))
        qxpool = ctx.enter_context(tc.tile_pool(name="qxpool", bufs=2))
        with tc.tile_pool(name="xpool", bufs=3) as xpool, tc.tile_pool(
            name="pps", bufs=2, space="PSUM"
        ) as pps, tc.tile_pool(name="shufps", bufs=2, space="PSUM") as shufps:

            def proj_side(x_dram, w_s, dest, lt, dma_eng, evict_act, xp, mainp, shufp):
                """Project one 512-wide l-tile of q or k into dest[:, p, ls]."""
                ls = slice(lt * LQ, (lt + 1) * LQ)
                x_s = xp.tile([128, NC, LQ], BF16, tag="x")
                xr = x_dram.rearrange("(c p) l -> p c l", p=128)[:, :, ls]
                if dma_eng is None:  # startup path: keep SP free for consts
                    dma_eng = nc.gpsimd
                dma_eng.dma_start(out=x_s[:], in_=xr)
                for p in range(NP):
                    psm = mainp.tile([128, LQ], F32, tag="psm")
                    for c in range(NC):
                        nc.tensor.matmul(
                            psm[:], w_s[:, c, p, :], x_s[:, c, :],
                            start=(c == 0), stop=(c == NC - 1),
                        )
                    km = ev.tile([128, LQ], BF16, tag="km")
                    if evict_act or p == 1:
                        nc.scalar.activation(out=km[:], in_=psm[:], func=ACT_COPY)
                    else:
                        nc.vector.tensor_copy(out=km[:], in_=psm[:])
                    pss = shufp.tile([128, LQ], F32, tag="psm" if shufp is mainp else "pss")
                    nc.tensor.matmul(pss[:], perm_s[:], km[:], start=True, stop=True)
                    t1 = ev.tile([128, LQ], BF16, tag="t1")
                    t2 = ev.tile([128, LQ], BF16, tag="t2")
                    # t1 (SBUF only) on Pool; t2 reads PSUM -> DVE; add on Pool
                    nc.gpsimd.tensor_mul(t1[:], km[:], cos_s[:, ls])
                    nc.vector.tensor_mul(t2[:], pss[:], sin_s[:, ls])
                    nc.gpsimd.tensor_add(dest[:, p, ls], t1[:], t2[:])

            for lt in range(L // LQ):
                # first xk tile loads on Pool in parallel with the SP consts
                proj_side(
                    xk, wkm_s, kcat, lt,
                    None if lt == 0 else nc.sync, True, xpool, pps, shufps,
                )
                # ---- v side ----
                ls = slice(lt * LQ, (lt + 1) * LQ)
                x_s = xpool.tile([128, NC, LQ], BF16, tag="x")
                nc.gpsimd.dma_start(
                    out=x_s[:], in_=xv.rearrange("(c p) l -> p c l", p=128)[:, :, ls]
                )
                for st in range(LQ // LK):
                    lk_i = lt * (LQ // LK) + st
                    psv = pps.tile([128, 256], F32, tag="psv")
                    for c in range(NC):
                        nc.tensor.matmul(
                            psv[:], x_s[:, c, st * LK : (st + 1) * LK], wv_s[:, c, :],
                            start=(c == 0), stop=(c == NC - 1),
                        )
                    nc.scalar.activation(
                        out=v_s[:, lk_i, :, 0:64],
                        in_=psv.rearrange("p (h d) -> p h d", h=HPC),
                        func=ACT_COPY,
                    )
            # late consts (not needed until phase B / C)
            nc.sync.dma_start(
                out=wqm_s[:], in_=wqm.rearrange("(c p) n m -> p c n m", p=128)
            )
            nc.scalar.dma_start(out=wo_s[:], in_=wo.rearrange("n p e -> p n e"))
            nc.scalar.dma_start(out=ident_s[:], in_=ident[:])

        # ================= phase B: attention sweep =================
        with tc.tile_pool(name="qkps", bufs=2, space="PSUM") as qkpool, tc.tile_pool(
            name="avps", bufs=1, space="PSUM"
        ) as avpool, tc.tile_pool(name="denps", bufs=1, space="PSUM") as denpool, tc.tile_pool(
            name="qpps", bufs=1, space="PSUM"
        ) as qpps, tc.tile_pool(name="ut", bufs=3) as utpool, tc.tile_pool(
            name="rec", bufs=2
        ) as recpool:
            # q projection for the first lq tile (PE-serial prologue)
            proj_side(xq, wqm_s, qcat, 0, nc.sync, False, qxpool, qpps, qpps)
            AVLAG = 4  # AV matmuls trail QK/exp by this many iterations
            for lq in range(NLQ):
                qs = slice(lq * LQ, (lq + 1) * LQ)
                av = avpool.tile([128, HPC, HPC, K], F32, tag="av")
                den = denpool.tile([128, HPC, HPC], F32, tag="den")
                uts = {}

                def do_av(idx):
                    lk, half = divmod(idx, NP)
                    ut = uts.pop(idx)
                    for hh in range(2):
                        h = half * 2 + hh
                        for c in range(HPC):
                            # av bank = 2 chunks; start zeroes the whole
                            # bank: only (h==0, even c) starts, the last
                            # matmul into the bank closes the group.
                            nc.tensor.matmul(
                                av[:, c, h, :],
                                ut[:, hh, c * 128 : (c + 1) * 128],
                                v_s[:, lk, h, 0:64],
                                start=(lk == 0 and h == 0 and c % 2 == 0),
                                stop=(lk == NLK - 1 and h == HPC - 1 and c % 2 == 1),
                            )
                        for c in range(HPC):
                            nc.tensor.matmul(
                                den[:, c, h : h + 1],
                                ut[:, hh, c * 128 : (c + 1) * 128],
                                v_s[:, lk, h, 64:65],
                                start=(lk == 0 and h == 0 and c == 0),
                                stop=(lk == NLK - 1 and h == HPC - 1 and c == HPC - 1),
                            )

                pending = []  # deferred y2+mult finishes for fast-exp tiles

                def do_av_h(idx):
                    lk, h = divmod(idx, HPC)
                    ut = uts.pop(idx)
                    for c in range(HPC):
                        nc.tensor.matmul(
                            av[:, c, h, :],
                            ut[:, c * 128 : (c + 1) * 128],
                            v_s[:, lk, h, 0:64],
                            start=(lk == 0 and h == 0 and c % 2 == 0),
                            stop=(lk == NLK - 1 and h == HPC - 1 and c % 2 == 1),
                        )
                    for c in range(HPC):
                        nc.tensor.matmul(
                            den[:, c, h : h + 1],
                            ut[:, c * 128 : (c + 1) * 128],
                            v_s[:, lk, h, 64:65],
                            start=(lk == 0 and h == 0 and c == 0),
                            stop=(lk == NLK - 1 and h == HPC - 1 and c == HPC - 1),
                        )

                def drain_tile():
                    rec = recpool.tile([128, HPC, HPC], F32, tag="rec")
                    nc.vector.reciprocal(out=rec[:], in_=den[:])
                    for c in range(HPC):
                        for h in range(HPC):
                            # av is PSUM: DVE or ACT only (not Pool)
                            if (c * HPC + h) % 2 == 0:
                                nc.vector.tensor_scalar_mul(
                                    attnN[:, lq * HPC + c, h, :],
                                    av[:, c, h, :],
                                    rec[:, c, h : h + 1],
                                )
                            else:
                                nc.scalar.activation(
                                    out=attnN[:, lq * HPC + c, h, :],
                                    in_=av[:, c, h, :], func=ACT_COPY,
                                    scale=rec[:, c, h : h + 1],
                                )

                def finish_fast():
                    eng, ut, y1, y2 = pending.pop(0)
                    # y2 = y1 - A*d: int-domain shift of the Schraudolph arg
                    eng.tensor_scalar(
                        out=y2[:], in0=y1[:],
                        scalar1=float(-round(_FE_A * _FE_D)), scalar2=None,
                        op0=mybir.AluOpType.add,
                    )
                    eng.tensor_mul(
                        ut[:], y1[:].bitcast(BF16), y2[:].bitcast(BF16)
                    )

                if _PER_HEAD:
                    AVLAG_H = int(os.environ.get("AVLAG_H", "20"))
                    for idx in range(NLK * HPC):
                        lk, h = divmod(idx, HPC)
                        ks = slice(lk * LK, (lk + 1) * LK)
                        strip = slice((h % 2) * 64, (h % 2) * 64 + 64)
                        qk = qkpool.tile([128, LQ], F32, tag="qk", bufs=int(os.environ.get("QKBUFS", "4")))
                        nc.tensor.matmul(
                            qk[:], kcat[strip, h // 2, ks], qcat[strip, h // 2, qs],
                            start=True, stop=True,
                            tile_position=((h % 2) * 64, 0),
                        )
                        ut = utpool.tile([128, LQ], BF16, tag="uth", bufs=AVLAG_H + 3)
                        uts[idx] = ut
                        eng_c = _exp_engine(lq * NLK * HPC + idx)
                        if eng_c == "A":
                            nc.scalar.activation(
                                out=ut[:], in_=qk[:], func=ACT_EXP,
                                scale=float(1.0 / np.sqrt(K)),
                            )
                        else:
                            # Pool (GPSIMD) cannot read PSUM on hw: y1 always
                            # on DVE; "P" tiles finish (y2+mult, SBUF-only)
                            # on Pool.
                            eng = nc.gpsimd if eng_c == "P" else nc.vector
                            sc = _FE_A / 16.0
                            y1 = utpool.tile([128, LQ], I16, tag=f"z1{eng_c}", bufs=3)
                            y2 = utpool.tile([128, LQ], I16, tag=f"z2{eng_c}", bufs=3)
                            nc.vector.tensor_scalar(
                                out=y1[:], in0=qk[:],
                                scalar1=sc, scalar2=float(_FE_B + _FE_A * _FE_D / 2),
                                op0=mybir.AluOpType.mult, op1=mybir.AluOpType.add,
                            )
                            pending.append((eng, ut, y1, y2))
                        if pending and (eng_c == "A" or len(pending) > 1):
                            finish_fast()
                        if idx >= AVLAG_H:
                            do_av_h(idx - AVLAG_H)
                    while pending:
                        finish_fast()
                    for idx in range(NLK * HPC - AVLAG_H, NLK * HPC):
                        do_av_h(idx)
                    drain_tile()
                    # q projection for the next lq tile
                    if lq + 1 < NLQ:
                        proj_side(xq, wqm_s, qcat, lq + 1, nc.sync, False, qxpool, qpps, qpps)
                    continue
                for idx in range(NLK * NP):
                    lk, half = divmod(idx, NP)
                    ks = slice(lk * LK, (lk + 1) * LK)
                    qk = qkpool.tile([128, 2, LQ], F32, tag="qk")
                    for hh in range(2):
                        strip = slice(hh * 64, (hh + 1) * 64)
                        nc.tensor.matmul(
                            qk[:, hh, :],
                            kcat[strip, half, ks],
                            qcat[strip, half, qs],
                            start=True, stop=True,
                            tile_position=(hh * 64, 0),
                        )
                    ut = utpool.tile([128, 2, LQ], BF16, tag="ut", bufs=AVLAG + 3)
                    uts[idx] = ut
                    it = lq * NLK * NP + idx
                    eng_c = _exp_engine(it)
                    if eng_c == "A":
                        # per-half ops free each qk half-region sooner,
                        # doubling the effective score-tile pipeline depth
                        if _HALF_EXP:
                            for hh in range(2):
                                nc.scalar.activation(
                                    out=ut[:, hh, :], in_=qk[:, hh, :], func=ACT_EXP,
                                    scale=float(1.0 / np.sqrt(K)),
                                )
                        else:
                            nc.scalar.activation(
                                out=ut[:], in_=qk[:], func=ACT_EXP,
                                scale=float(1.0 / np.sqrt(K)),
                            )
                    else:
                        eng = nc.gpsimd if eng_c == "P" else nc.vector
                        sc = _FE_A / 16.0  # A/2 folded with the 1/sqrt(64)
                        y1 = utpool.tile([128, 2, LQ], I16, tag=f"y1{eng_c}", bufs=3)
                        y2 = utpool.tile([128, 2, LQ], I16, tag=f"y2{eng_c}", bufs=3)
                        # y1 (the only reader of the qk PSUM tile) is emitted
                        # now (per half) so it isn't queued behind other tiles'
                        # finish ops on this engine; y2+mult are deferred.
                        if _HALF_EXP:
                            for hh in range(2):
                                nc.vector.tensor_scalar(
                                    out=y1[:, hh, :], in0=qk[:, hh, :],
                                    scalar1=sc, scalar2=float(_FE_B + _FE_A * _FE_D / 2),
                                    op0=mybir.AluOpType.mult, op1=mybir.AluOpType.add,
                                )
                        else:
                            nc.vector.tensor_scalar(
                                out=y1[:], in0=qk[:],
                                scalar1=sc, scalar2=float(_FE_B + _FE_A * _FE_D / 2),
                                op0=mybir.AluOpType.mult, op1=mybir.AluOpType.add,
                            )
                        pending.append((eng, ut, y1, y2))
                    if pending and (eng_c == "A" or len(pending) > 1):
                        finish_fast()
                    if idx >= AVLAG:
                        do_av(idx - AVLAG)
                while pending:
                    finish_fast()
                for idx in range(NLK * NP - AVLAG, NLK * NP):
                    do_av(idx)
                # q projection for the next lq tile (hidden in the sweep's
                # PE slack; B is exp-latency-bound, not PE-bound)
                if lq + 1 < NLQ:
                    proj_side(xq, wqm_s, qcat, lq + 1, nc.sync, False, qxpool, qpps, qpps)
                # drain: reciprocal of denominators + normalize-evict
                drain_tile()

        # ================= phase C: transpose + o-projection =================
        with tc.tile_pool(name="trps", bufs=4, space="PSUM") as trpool, tc.tile_pool(
            name="ops", bufs=4, space="PSUM"
        ) as opool, tc.tile_pool(name="aT", bufs=3) as atpool, tc.tile_pool(
            name="oev", bufs=6
        ) as oev:
            for i in range(NLQC):
                aT = atpool.tile([128, NP, 128], BF16, tag="aT")
                for c2 in range(NP):
                    trp = trpool.tile([128, 128], BF16, tag="trp")
                    nc.tensor.transpose(
                        trp[:], attnN[:, i, 2 * c2 : 2 * c2 + 2, :], ident_s[:]
                    )
                    nc.scalar.activation(out=aT[:, c2, :], in_=trp[:], func=ACT_COPY)
                ot = oev.tile([128, D], BF16, tag="ot")
                for et in range(D // LQ):
                    es = slice(et * LQ, (et + 1) * LQ)
                    po = opool.tile([128, LQ], F32, tag="po")
                    for c2 in range(NP):
                        nc.tensor.matmul(
                            po[:], aT[:, c2, :], wo_s[:, c2, es],
                            start=(c2 == 0), stop=(c2 == NP - 1),
                        )
                    # po is PSUM: DVE or ACT only (not Pool)
                    if et == 0:
                        nc.vector.tensor_copy(out=ot[:, es], in_=po[:])
                    else:
                        nc.scalar.activation(out=ot[:, es], in_=po[:], func=ACT_COPY)
                (nc.sync if i % 2 == 0 else nc.gpsimd).dma_start(
                    out=out[i * LK : (i + 1) * LK, :], in_=ot[:]
                )

    nc.compile()
    return nc


# ---------------- host side ----------------


def _perm_matrix():
    """P such that P^T @ x applies rotate_half on rot strips, zeroes pass."""
    P = np.zeros((128, 128), np.float32)
    for base in (0, 64):
        for i in range(16):
            P[base + 2 * i + 1, base + 2 * i] = -1.0
            P[base + 2 * i, base + 2 * i + 1] = 1.0
    return P


def make_in_maps(query, key, value, rot_pos_emb, q_kernel, k_kernel, v_kernel, o_kernel, L=L_FULL):
    f = np.asarray(rot_pos_emb, np.float32)
    cosT = np.cos(f).T.astype(np.float32)  # (32, L)
    sinT = np.sin(f).T.astype(np.float32)
    ones = np.ones((32, L), np.float32)
    zeros = np.zeros((32, L), np.float32)
    costile = np.concatenate([cosT, ones, cosT, ones], 0).astype(NPBF)
    sintile = np.concatenate([sinT, zeros, sinT, zeros], 0).astype(NPBF)
    permM = _perm_matrix().astype(NPBF)
    identM = np.eye(128, dtype=np.float32).astype(NPBF)

    def main_w(wk, hs):
        # [D, NP, 128]: pair p cols = [rot(h0) | pass(h0) | rot(h1) | pass(h1)]
        cols = []
        for p in range(NP):
            h0, h1 = hs[2 * p], hs[2 * p + 1]
            cols.append(
                np.concatenate(
                    [wk[:, h0, :ROT], wk[:, h0, ROT:], wk[:, h1, :ROT], wk[:, h1, ROT:]],
                    axis=1,
                )
            )
        return np.stack(cols, 1).astype(NPBF)  # (D, NP, 128)

    qk_ = np.asarray(q_kernel, np.float32)
    kk_ = np.asarray(k_kernel, np.float32)
    vk = np.asarray(v_kernel, np.float32)
    ok = np.asarray(o_kernel, np.float32)

    in_maps = []
    for core in range(NCORES):
        b, g = divmod(core, NCORES // B)
        hs = list(range(g * HPC, (g + 1) * HPC))
        m = {
            "xq": np.ascontiguousarray(np.asarray(query[b], np.float32).T).astype(NPBF),
            "xk": np.ascontiguousarray(np.asarray(key[b], np.float32).T).astype(NPBF),
            "xv": np.ascontiguousarray(np.asarray(value[b], np.float32).T).astype(NPBF),
            "cost": costile,
            "sint": sintile,
            "perm": permM,
            "ident": identM,
            "wqm": main_w(qk_, hs),
            "wkm": main_w(kk_, hs),
            "wv": np.ascontiguousarray(
                np.concatenate([vk[:, h, :] for h in hs], axis=1)
            ).astype(NPBF),
            "wo": np.stack(
                [ok[hs[0:2]].reshape(128, D), ok[hs[2:4]].reshape(128, D)], 0
            ).astype(NPBF),
        }
        in_maps.append(m)
    return in_maps


_CACHED = {}


def kernel(query, key, value, rot_pos_emb, q_kernel, k_kernel, v_kernel, o_kernel):
    from concourse.bass_utils import run_bass_kernel_spmd

    if "nc" not in _CACHED:
        _CACHED["nc"] = build_nc(L_FULL)
    nc = _CACHED["nc"]
    in_maps = make_in_maps(
        query, key, value, rot_pos_emb, q_kernel, k_kernel, v_kernel, o_kernel
    )
    res = run_bass_kernel_spmd(nc, in_maps, core_ids=list(range(NCORES)))
    outs = res.results
    full = np.zeros((B, L_FULL, D), np.float32)
    for core in range(NCORES):
        b = core // (NCORES // B)
        full[b] += outs[core]["out"]
    return full


# revision 54
# speedup vs baseline: 1.0089x; 1.0089x over previous
"""MultiHeadAttention (partial RoPE) Trainium2 Bass kernel — v3.

Sharding: 8 cores = 2 batches x 4 head-groups (4 heads each).
Each core computes a partial output (L, D) for its batch from its 4 heads;
the host sums the 4 partials per batch (the "all-reduce after o-proj").

Design (vs the naive decomposition; the PE cost is ~N output columns per
matmul regardless of contraction rows C or lhsT width M):
  - q/k stored as qcat/kcat [128 = 2 heads x (32 rot + 32 pass), pair, L]:
    QK^T per head is ONE C=64 matmul (rot+pass contracted together).
  - rotate_half = a PE permutation matmul (perm [128,128]) on the projected
    q/k instead of a second full D-contraction projection.
  - AV in natural orientation: out[lq-chunk 128, d] with lhsT = exp scores
    [lk, lq-chunk], rhs = v_h [lk, 64]: N=64 per matmul (vs N=512, M=65);
    softmax denominators via extra N=1 matmuls against a ones column.
  - denominators land as per-PARTITION scalars -> normalize is a cheap
    tensor_scalar/ACT-scale multiply (no DRAM round-trip / broadcast).
  - attn transposed to [hd, lq] via PE transpose (N=128) so the
    o-projection contracts C=128 (2 heads stacked).
  - softmax exp is the ACT-engine floor (~110us): ~43% of score tiles are
    computed instead on DVE/Pool with a refined Schraudolph fast-exp
    (int16(A*x+B) bitcast to bf16; product of a phase-shifted pair,
    ~+-1.9% rel err): y1 (PSUM read) on DVE, y2+mult on Pool/DVE.
    GPSIMD/Pool cannot access PSUM on hw - only DVE/ACT ops read PSUM.
  - per-head 1-bank PSUM score tiles (4 in flight) + AV matmuls lagged 20
    head-iterations behind QK/exp: hides exp latency; the kernel ends up
    PE-bound at ~84% occupancy.
  - phases: A = k/v projections; B = sweep (q projection for the next lq
    tile hidden inside); C = transpose + o-projection + bf16 output DMA.
"""

import sys

if "/opt/trn_rl_repo" not in sys.path:
    sys.path.insert(0, "/opt/trn_rl_repo")

from contextlib import ExitStack

import ml_dtypes
import numpy as np

import concourse.bass as bass
import concourse.mybir as mybir
from concourse import bacc
import concourse.tile as tile

B, L_FULL, D = 2, 2048, 1024
H, K = 16, 64
ROT = 32
HPC = 4  # heads per core
NP = 2  # head pairs per core
NCORES = 8

F32 = mybir.dt.float32
BF16 = mybir.dt.bfloat16
I16 = mybir.dt.int16
NPBF = ml_dtypes.bfloat16

LQ = 512  # lq tile
LK = 128  # lk tile (partition tile)
NC = D // 128  # contraction chunks for projections

# fast-exp (Schraudolph in bf16-bit domain, refined by a phase-shifted pair):
#   exp(x) ~= S((x+d)/2) * S((x-d)/2),  S(t) = bitcast_bf16(int16(A*t + B))
# ~+-1.9% relative error; used to offload part of the softmax exp from the
# ACT engine (the kernel's bottleneck) onto the otherwise-idle Pool/DVE.
_LOG2E = 1.4426950408889634
_FE_C = 0.054
_FE_D = 0.35
_FE_A = 2.0**7 * _LOG2E
_FE_B = 127.0 * 2**7 - _FE_C * 2**7
# exp engine shares per 128 iterations (ACT is ~2.5x faster per element
# than the 3-op fast-exp path on Pool/DVE); interleaved evenly.
import os

_EXP_SHARES = {
    "A": int(os.environ.get("EXP_A", "72")),
    "P": int(os.environ.get("EXP_P", "30")),
    "D": int(os.environ.get("EXP_D", "26")),
}
_HALF_EXP = os.environ.get("HALF_EXP", "0") == "1"
_PER_HEAD = os.environ.get("PER_HEAD", "1") == "1"
_PH_SHARES = {
    "A": int(os.environ.get("PH_A", "146")),
    "P": int(os.environ.get("PH_P", "74")),
    "D": int(os.environ.get("PH_D", "36")),
}


def _make_exp_sched(shares):
    tot = sum(shares.values())
    acc = {e: 0.0 for e in shares}
    sched = []
    for _ in range(tot):
        for e in acc:
            acc[e] += shares[e] / tot
        pick = max(acc, key=lambda e: acc[e])
        acc[pick] -= 1.0
        sched.append(pick)
    return sched


_EXP_SCHED = _make_exp_sched(_PH_SHARES if _PER_HEAD else _EXP_SHARES)


def _exp_engine(it):
    return _EXP_SCHED[it % len(_EXP_SCHED)]


def build_nc(L=L_FULL):
    """Build the single-core SPMD program. Returns nc."""
    nc = bacc.Bacc("TRN2", target_bir_lowering=False)

    NLQ = L // LQ  # lq tiles
    NLK = L // LK  # lk tiles
    NLQC = L // LK  # lq chunks (128)

    ACT_COPY = mybir.ActivationFunctionType.Copy
    ACT_EXP = mybir.ActivationFunctionType.Exp

    # ---- DRAM I/O (per-core shapes) ----
    xq = nc.dram_tensor("xq", [D, L], BF16, kind="ExternalInput")  # query[b].T
    xk = nc.dram_tensor("xk", [D, L], BF16, kind="ExternalInput")
    xv = nc.dram_tensor("xv", [D, L], BF16, kind="ExternalInput")
    wqm = nc.dram_tensor("wqm", [D, NP, 128], BF16, kind="ExternalInput")
    wkm = nc.dram_tensor("wkm", [D, NP, 128], BF16, kind="ExternalInput")
    wv = nc.dram_tensor("wv", [D, 256], BF16, kind="ExternalInput")
    wo = nc.dram_tensor("wo", [NP, 128, D], BF16, kind="ExternalInput")
    cost = nc.dram_tensor("cost", [128, L], BF16, kind="ExternalInput")
    sint = nc.dram_tensor("sint", [128, L], BF16, kind="ExternalInput")
    perm = nc.dram_tensor("perm", [128, 128], BF16, kind="ExternalInput")
    ident = nc.dram_tensor("ident", [128, 128], BF16, kind="ExternalInput")
    out = nc.dram_tensor("out", [L, D], BF16, kind="ExternalOutput")

    with tile.TileContext(nc) as tc, ExitStack() as ctx:
        consts = ctx.enter_context(tc.tile_pool(name="consts", bufs=1))
        persist = ctx.enter_context(tc.tile_pool(name="persist", bufs=1))

        # ---- load k-side consts + first x tiles up front, in parallel across
        # engines, so the PE can start ASAP. wqm/wo/ident are emitted at the
        # end of phase A so they don't block phase-A ACT evictions. ----
        wkm_s = consts.tile([128, NC, NP, 128], BF16, tag="wkm")
        nc.sync.dma_start(out=wkm_s[:], in_=wkm.rearrange("(c p) n m -> p c n m", p=128))
        perm_s = consts.tile([128, 128], BF16, tag="perm")
        nc.sync.dma_start(out=perm_s[:], in_=perm[:])
        xk0_s = consts.tile([128, NC, LQ], BF16, tag="xk0")
        _xk0r = xk.rearrange("(c p) l -> p c l", p=128)[:, :, 0:LQ]
        nc.gpsimd.dma_start(out=xk0_s[:, 0:4, :], in_=_xk0r[:, 0:4, :])
        nc.scalar.dma_start(out=xk0_s[:, 4:8, :], in_=_xk0r[:, 4:8, :])
        wv_s = consts.tile([128, NC, 256], BF16, tag="wv")
        nc.scalar.dma_start(out=wv_s[:], in_=wv.rearrange("(c p) n -> p c n", p=128))
        cos_s = consts.tile([128, L], BF16, tag="cos")
        nc.scalar.dma_start(out=cos_s[:], in_=cost[:])
        sin_s = consts.tile([128, L], BF16, tag="sin")
        nc.scalar.dma_start(out=sin_s[:], in_=sint[:])
        wqm_s = consts.tile([128, NC, NP, 128], BF16, tag="wqm")
        wo_s = consts.tile([128, NP, D], BF16, tag="wo")
        ident_s = consts.tile([128, 128], BF16, tag="ident")

        # ---- persistent activations ----
        qcat = persist.tile([128, NP, L], BF16, tag="qcat")
        kcat = persist.tile([128, NP, L], BF16, tag="kcat")
        v_s = persist.tile([128, NLK, HPC, 65], BF16, tag="v")
        attnN = persist.tile([128, NLQC, HPC, K], BF16, tag="attnN")
        aTall = persist.tile([128, NLQC, NP, 128], BF16, tag="aTall")
        nc.vector.memset(v_s[:, :, :, 64:65], 1.0)

        # ================= phase A: k/v projections =================
        # (q projection runs inside the phase-B sweep, hidden under the
        # ACT-bound exp pipeline, using a single shared PSUM bank.)
        ev = ctx.enter_context(tc.tile_pool(name="ev", bufs=3))
        qxpool = ctx.enter_context(tc.tile_pool(name="qxpool", bufs=2))
        with tc.tile_pool(name="xpool", bufs=3) as xpool, tc.tile_pool(
            name="pps", bufs=2, space="PSUM"
        ) as pps, tc.tile_pool(name="shufps", bufs=2, space="PSUM") as shufps:

            def proj_side(x_dram, w_s, dest, lt, dma_eng, evict_act, xp, mainp, shufp):
                """Project one 512-wide l-tile of q or k into dest[:, p, ls]."""
                ls = slice(lt * LQ, (lt + 1) * LQ)
                if dma_eng is None:  # startup path: tile preloaded above
                    x_s = xk0_s
                else:
                    x_s = xp.tile([128, NC, LQ], BF16, tag="x")
                    xr = x_dram.rearrange("(c p) l -> p c l", p=128)[:, :, ls]
                    dma_eng.dma_start(out=x_s[:], in_=xr)
                for p in range(NP):
                    psm = mainp.tile([128, LQ], F32, tag="psm")
                    for c in range(NC):
                        nc.tensor.matmul(
                            psm[:], w_s[:, c, p, :], x_s[:, c, :],
                            start=(c == 0), stop=(c == NC - 1),
                        )
                    km = ev.tile([128, LQ], BF16, tag="km")
                    nc.scalar.activation(out=km[:], in_=psm[:], func=ACT_COPY)
                    pss = shufp.tile([128, LQ], F32, tag="psm" if shufp is mainp else "pss")
                    nc.tensor.matmul(pss[:], perm_s[:], km[:], start=True, stop=True)
                    t1 = ev.tile([128, LQ], BF16, tag="t1")
                    t2 = ev.tile([128, LQ], BF16, tag="t2")
                    # t1 (SBUF only) on Pool; t2 reads PSUM -> DVE; add on Pool
                    nc.gpsimd.tensor_mul(t1[:], km[:], cos_s[:, ls])
                    nc.vector.tensor_mul(t2[:], pss[:], sin_s[:, ls])
                    nc.gpsimd.tensor_add(dest[:, p, ls], t1[:], t2[:])

            for lt in range(L // LQ):
                # first xk tile loads on Pool in parallel with the SP consts
                proj_side(
                    xk, wkm_s, kcat, lt,
                    None if lt == 0 else nc.sync, True, xpool, pps, shufps,
                )
                # ---- v side ----
                ls = slice(lt * LQ, (lt + 1) * LQ)
                x_s = xpool.tile([128, NC, LQ], BF16, tag="x")
                nc.gpsimd.dma_start(
                    out=x_s[:], in_=xv.rearrange("(c p) l -> p c l", p=128)[:, :, ls]
                )
                for st in range(LQ // LK):
                    lk_i = lt * (LQ // LK) + st
                    psv = pps.tile([128, 256], F32, tag="psv")
                    for c in range(NC):
                        nc.tensor.matmul(
                            psv[:], x_s[:, c, st * LK : (st + 1) * LK], wv_s[:, c, :],
                            start=(c == 0), stop=(c == NC - 1),
                        )
                    nc.scalar.activation(
                        out=v_s[:, lk_i, :, 0:64],
                        in_=psv.rearrange("p (h d) -> p h d", h=HPC),
                        func=ACT_COPY,
                    )
            # late consts (not needed until phase B / C)
            nc.sync.dma_start(
                out=wqm_s[:], in_=wqm.rearrange("(c p) n m -> p c n m", p=128)
            )
            nc.scalar.dma_start(out=wo_s[:], in_=wo.rearrange("n p e -> p n e"))
            nc.scalar.dma_start(out=ident_s[:], in_=ident[:])

        # ================= phase B: attention sweep =================
        trpool_box = {}

        def make_tr(i):
            def f():
                for c2 in range(NP):
                    trp = trpool_box["p"].tile([128, 128], BF16, tag="trp")
                    nc.tensor.transpose(
                        trp[:], attnN[:, i, 2 * c2 : 2 * c2 + 2, :], ident_s[:]
                    )
                    nc.scalar.activation(
                        out=aTall[:, i, c2, :], in_=trp[:], func=ACT_COPY
                    )
            return f

        extraC = []
        with tc.tile_pool(name="qkps", bufs=2, space="PSUM") as qkpool, tc.tile_pool(
            name="avps", bufs=1, space="PSUM"
        ) as avpool, tc.tile_pool(name="denps", bufs=1, space="PSUM") as denpool, tc.tile_pool(
            name="ut", bufs=3
        ) as utpool, tc.tile_pool(name="rec", bufs=2) as recpool:
            qpps = tc.alloc_tile_pool(name="qpps", bufs=1, space="PSUM")
            # q projection for the first lq tile (PE-serial prologue)
            proj_side(xq, wqm_s, qcat, 0, nc.sync, False, qxpool, qpps, qpps)
            AVLAG = 4  # AV matmuls trail QK/exp by this many iterations
            for lq in range(NLQ):
                qs = slice(lq * LQ, (lq + 1) * LQ)
                av = avpool.tile([128, HPC, HPC, K], F32, tag="av")
                den = denpool.tile([128, HPC, HPC], F32, tag="den")
                uts = {}

                def do_av(idx):
                    lk, half = divmod(idx, NP)
                    ut = uts.pop(idx)
                    for hh in range(2):
                        h = half * 2 + hh
                        for c in range(HPC):
                            # av bank = 2 chunks; start zeroes the whole
                            # bank: only (h==0, even c) starts, the last
                            # matmul into the bank closes the group.
                            nc.tensor.matmul(
                                av[:, c, h, :],
                                ut[:, hh, c * 128 : (c + 1) * 128],
                                v_s[:, lk, h, 0:64],
                                start=(lk == 0 and h == 0 and c % 2 == 0),
                                stop=(lk == NLK - 1 and h == HPC - 1 and c % 2 == 1),
                            )
                        for c in range(HPC):
                            nc.tensor.matmul(
                                den[:, c, h : h + 1],
                                ut[:, hh, c * 128 : (c + 1) * 128],
                                v_s[:, lk, h, 64:65],
                                start=(lk == 0 and h == 0 and c == 0),
                                stop=(lk == NLK - 1 and h == HPC - 1 and c == HPC - 1),
                            )

                pending = []  # deferred y2+mult finishes for fast-exp tiles

                def do_av_h(idx):
                    lk, h = divmod(idx, HPC)
                    ut = uts.pop(idx)
                    for c in range(HPC):
                        nc.tensor.matmul(
                            av[:, c, h, :],
                            ut[:, c * 128 : (c + 1) * 128],
                            v_s[:, lk, h, 0:64],
                            start=(lk == 0 and h == 0 and c % 2 == 0),
                            stop=(lk == NLK - 1 and h == HPC - 1 and c % 2 == 1),
                        )
                    for c in range(HPC):
                        nc.tensor.matmul(
                            den[:, c, h : h + 1],
                            ut[:, c * 128 : (c + 1) * 128],
                            v_s[:, lk, h, 64:65],
                            start=(lk == 0 and h == 0 and c == 0),
                            stop=(lk == NLK - 1 and h == HPC - 1 and c == HPC - 1),
                        )

                def drain_tile():
                    rec = recpool.tile([128, HPC, HPC], F32, tag="rec")
                    nc.vector.reciprocal(out=rec[:], in_=den[:])
                    for c in range(HPC):
                        for h in range(HPC):
                            # av is PSUM: DVE or ACT only (not Pool)
                            if (c * HPC + h) % 2 == 0:
                                nc.vector.tensor_scalar_mul(
                                    attnN[:, lq * HPC + c, h, :],
                                    av[:, c, h, :],
                                    rec[:, c, h : h + 1],
                                )
                            else:
                                nc.scalar.activation(
                                    out=attnN[:, lq * HPC + c, h, :],
                                    in_=av[:, c, h, :], func=ACT_COPY,
                                    scale=rec[:, c, h : h + 1],
                                )

                def finish_fast():
                    eng, ut, y1, y2 = pending.pop(0)
                    # y2 = y1 - A*d: int-domain shift of the Schraudolph arg
                    eng.tensor_scalar(
                        out=y2[:], in0=y1[:],
                        scalar1=float(-round(_FE_A * _FE_D)), scalar2=None,
                        op0=mybir.AluOpType.add,
                    )
                    eng.tensor_mul(
                        ut[:], y1[:].bitcast(BF16), y2[:].bitcast(BF16)
                    )

                if _PER_HEAD:
                    AVLAG_H = int(os.environ.get("AVLAG_H", "20"))
                    for idx in range(NLK * HPC):
                        lk, h = divmod(idx, HPC)
                        ks = slice(lk * LK, (lk + 1) * LK)
                        strip = slice((h % 2) * 64, (h % 2) * 64 + 64)
                        qk = qkpool.tile([128, LQ], F32, tag="qk", bufs=int(os.environ.get("QKBUFS", "4")))
                        nc.tensor.matmul(
                            qk[:], kcat[strip, h // 2, ks], qcat[strip, h // 2, qs],
                            start=True, stop=True,
                            tile_position=((h % 2) * 64, 0),
                        )
                        ut = utpool.tile([128, LQ], BF16, tag="uth", bufs=AVLAG_H + 3)
                        uts[idx] = ut
                        eng_c = _exp_engine(lq * NLK * HPC + idx)
                        if eng_c == "A":
                            nc.scalar.activation(
                                out=ut[:], in_=qk[:], func=ACT_EXP,
                                scale=float(1.0 / np.sqrt(K)),
                            )
                        else:
                            # Pool (GPSIMD) cannot read PSUM on hw: y1 always
                            # on DVE; "P" tiles finish (y2+mult, SBUF-only)
                            # on Pool.
                            eng = nc.gpsimd if eng_c == "P" else nc.vector
                            sc = _FE_A / 16.0
                            y1 = utpool.tile([128, LQ], I16, tag=f"z1{eng_c}", bufs=3)
                            y2 = utpool.tile([128, LQ], I16, tag=f"z2{eng_c}", bufs=3)
                            nc.vector.tensor_scalar(
                                out=y1[:], in0=qk[:],
                                scalar1=sc, scalar2=float(_FE_B + _FE_A * _FE_D / 2),
                                op0=mybir.AluOpType.mult, op1=mybir.AluOpType.add,
                            )
                            pending.append((eng, ut, y1, y2))
                        if pending and (eng_c == "A" or len(pending) > 1):
                            finish_fast()
                        if idx >= AVLAG_H:
                            do_av_h(idx - AVLAG_H)
                        if extraC and idx % 2 == 0:
                            extraC.pop(0)()
                    while pending:
                        finish_fast()
                    for idx in range(NLK * HPC - AVLAG_H, NLK * HPC):
                        do_av_h(idx)
                    drain_tile()
                    # q projection for the next lq tile
                    if lq + 1 < NLQ:
                        proj_side(xq, wqm_s, qcat, lq + 1, nc.sync, False, qxpool, qpps, qpps)
                    if lq == NLQ - 2:
                        # last q projection emitted: hand its PSUM bank to the
                        # transpose pool and stream chunks 0..11's transposes
                        # through the final sweep's slack
                        qpps.release()
                        trpool_box["p"] = tc.alloc_tile_pool(
                            name="trps", bufs=1, space="PSUM", side="right"
                        )
                        for i in range((NLQ - 1) * HPC):
                            extraC.append(make_tr(i))
                    continue
                for idx in range(NLK * NP):
                    lk, half = divmod(idx, NP)
                    ks = slice(lk * LK, (lk + 1) * LK)
                    qk = qkpool.tile([128, 2, LQ], F32, tag="qk")
                    for hh in range(2):
                        strip = slice(hh * 64, (hh + 1) * 64)
                        nc.tensor.matmul(
                            qk[:, hh, :],
                            kcat[strip, half, ks],
                            qcat[strip, half, qs],
                            start=True, stop=True,
                            tile_position=(hh * 64, 0),
                        )
                    ut = utpool.tile([128, 2, LQ], BF16, tag="ut", bufs=AVLAG + 3)
                    uts[idx] = ut
                    it = lq * NLK * NP + idx
                    eng_c = _exp_engine(it)
                    if eng_c == "A":
                        # per-half ops free each qk half-region sooner,
                        # doubling the effective score-tile pipeline depth
                        if _HALF_EXP:
                            for hh in range(2):
                                nc.scalar.activation(
                                    out=ut[:, hh, :], in_=qk[:, hh, :], func=ACT_EXP,
                                    scale=float(1.0 / np.sqrt(K)),
                                )
                        else:
                            nc.scalar.activation(
                                out=ut[:], in_=qk[:], func=ACT_EXP,
                                scale=float(1.0 / np.sqrt(K)),
                            )
                    else:
                        eng = nc.gpsimd if eng_c == "P" else nc.vector
                        sc = _FE_A / 16.0  # A/2 folded with the 1/sqrt(64)
                        y1 = utpool.tile([128, 2, LQ], I16, tag=f"y1{eng_c}", bufs=3)
                        y2 = utpool.tile([128, 2, LQ], I16, tag=f"y2{eng_c}", bufs=3)
                        # y1 (the only reader of the qk PSUM tile) is emitted
                        # now (per half) so it isn't queued behind other tiles'
                        # finish ops on this engine; y2+mult are deferred.
                        if _HALF_EXP:
                            for hh in range(2):
                                nc.vector.tensor_scalar(
                                    out=y1[:, hh, :], in0=qk[:, hh, :],
                                    scalar1=sc, scalar2=float(_FE_B + _FE_A * _FE_D / 2),
                                    op0=mybir.AluOpType.mult, op1=mybir.AluOpType.add,
                                )
                        else:
                            nc.vector.tensor_scalar(
                                out=y1[:], in0=qk[:],
                                scalar1=sc, scalar2=float(_FE_B + _FE_A * _FE_D / 2),
                                op0=mybir.AluOpType.mult, op1=mybir.AluOpType.add,
                            )
                        pending.append((eng, ut, y1, y2))
                    if pending and (eng_c == "A" or len(pending) > 1):
                        finish_fast()
                    if idx >= AVLAG:
                        do_av(idx - AVLAG)
                while pending:
                    finish_fast()
                for idx in range(NLK * NP - AVLAG, NLK * NP):
                    do_av(idx)
                # q projection for the next lq tile (hidden in the sweep's
                # PE slack; B is exp-latency-bound, not PE-bound)
                if lq + 1 < NLQ:
                    proj_side(xq, wqm_s, qcat, lq + 1, nc.sync, False, qxpool, qpps, qpps)
                # drain: reciprocal of denominators + normalize-evict
                drain_tile()

        # ================= phase C: remaining transposes + o-projection ======
        with tc.tile_pool(name="ops", bufs=4, space="PSUM") as opool, tc.tile_pool(
            name="oev", bufs=6
        ) as oev:
            for i in range((NLQ - 1) * HPC, NLQC):
                make_tr(i)()
            for i in range(NLQC):
                ot = oev.tile([128, D], BF16, tag="ot")
                for et in range(D // LQ):
                    es = slice(et * LQ, (et + 1) * LQ)
                    po = opool.tile([128, LQ], F32, tag="po")
                    for c2 in range(NP):
                        nc.tensor.matmul(
                            po[:], aTall[:, i, c2, :], wo_s[:, c2, es],
                            start=(c2 == 0), stop=(c2 == NP - 1),
                        )
                    # po is PSUM: DVE or ACT only (not Pool)
                    if et == 0:
                        nc.vector.tensor_copy(out=ot[:, es], in_=po[:])
                    else:
                        nc.scalar.activation(out=ot[:, es], in_=po[:], func=ACT_COPY)
                (nc.sync if i % 2 == 0 else nc.gpsimd).dma_start(
                    out=out[i * LK : (i + 1) * LK, :], in_=ot[:]
                )
            trpool_box["p"].release()

    nc.compile()
    return nc


# ---------------- host side ----------------


def _perm_matrix():
    """P such that P^T @ x applies rotate_half on rot strips, zeroes pass."""
    P = np.zeros((128, 128), np.float32)
    for base in (0, 64):
        for i in range(16):
            P[base + 2 * i + 1, base + 2 * i] = -1.0
            P[base + 2 * i, base + 2 * i + 1] = 1.0
    return P


def make_in_maps(query, key, value, rot_pos_emb, q_kernel, k_kernel, v_kernel, o_kernel, L=L_FULL):
    f = np.asarray(rot_pos_emb, np.float32)
    cosT = np.cos(f).T.astype(np.float32)  # (32, L)
    sinT = np.sin(f).T.astype(np.float32)
    ones = np.ones((32, L), np.float32)
    zeros = np.zeros((32, L), np.float32)
    costile = np.concatenate([cosT, ones, cosT, ones], 0).astype(NPBF)
    sintile = np.concatenate([sinT, zeros, sinT, zeros], 0).astype(NPBF)
    permM = _perm_matrix().astype(NPBF)
    identM = np.eye(128, dtype=np.float32).astype(NPBF)

    def main_w(wk, hs):
        # [D, NP, 128]: pair p cols = [rot(h0) | pass(h0) | rot(h1) | pass(h1)]
        cols = []
        for p in range(NP):
            h0, h1 = hs[2 * p], hs[2 * p + 1]
            cols.append(
                np.concatenate(
                    [wk[:, h0, :ROT], wk[:, h0, ROT:], wk[:, h1, :ROT], wk[:, h1, ROT:]],
                    axis=1,
                )
            )
        return np.stack(cols, 1).astype(NPBF)  # (D, NP, 128)

    qk_ = np.asarray(q_kernel, np.float32)
    kk_ = np.asarray(k_kernel, np.float32)
    vk = np.asarray(v_kernel, np.float32)
    ok = np.asarray(o_kernel, np.float32)

    in_maps = []
    for core in range(NCORES):
        b, g = divmod(core, NCORES // B)
        hs = list(range(g * HPC, (g + 1) * HPC))
        m = {
            "xq": np.ascontiguousarray(np.asarray(query[b], np.float32).T).astype(NPBF),
            "xk": np.ascontiguousarray(np.asarray(key[b], np.float32).T).astype(NPBF),
            "xv": np.ascontiguousarray(np.asarray(value[b], np.float32).T).astype(NPBF),
            "cost": costile,
            "sint": sintile,
            "perm": permM,
            "ident": identM,
            "wqm": main_w(qk_, hs),
            "wkm": main_w(kk_, hs),
            "wv": np.ascontiguousarray(
                np.concatenate([vk[:, h, :] for h in hs], axis=1)
            ).astype(NPBF),
            "wo": np.stack(
                [ok[hs[0:2]].reshape(128, D), ok[hs[2:4]].reshape(128, D)], 0
            ).astype(NPBF),
        }
        in_maps.append(m)
    return in_maps


_CACHED = {}


def kernel(query, key, value, rot_pos_emb, q_kernel, k_kernel, v_kernel, o_kernel):
    from concourse.bass_utils import run_bass_kernel_spmd

    if "nc" not in _CACHED:
        _CACHED["nc"] = build_nc(L_FULL)
    nc = _CACHED["nc"]
    in_maps = make_in_maps(
        query, key, value, rot_pos_emb, q_kernel, k_kernel, v_kernel, o_kernel
    )
    res = run_bass_kernel_spmd(nc, in_maps, core_ids=list(range(NCORES)))
    outs = res.results
    full = np.zeros((B, L_FULL, D), np.float32)
    for core in range(NCORES):
        b = core // (NCORES // B)
        full[b] += outs[core]["out"]
    return full
